# revision 25
# baseline (speedup 1.0000x reference)
"""GroupedEmbeddingBag Trainium2 kernel.

Problem: T=8 tables of [N=200000, D=128] f32, per table L=163840 indices
pooled (sum) into B=8192 bags via CSR offsets. Output [B, T*D].

Sharding: table-wise — core t owns table t end-to-end (gather + pool).

The end-to-end metric here is dominated by host<->device transfer over the
axon tunnel (~35-75 MB/s), so the kernel minimizes wire bytes:
  - host dedups each table to its referenced rows (~112k of 200k) and
    remaps indices;
  - the deduped table ships 7-bit quantized (symmetric, one global
    compile-time scale DELTA; rel-err budget is 2e-2, 7-bit costs ~8e-3),
    plane-packed as 4+2+1 bit planes into 112 bytes/row and unpacked on
    the DVE with fused shift+and ops;
  - the local bag id rides in bits 18-24 of the index tensor; iota is
    generated on device;
  - output ships as bf16.

Device algorithm per core:
  - Host lays out the L indices as [128, 1280] "chunk" columns
    (chunk c = index positions [128c, 128c+128), lane p = position 128c+p).
  - Greedy variable-length windows of consecutive chunks, extended while
    every table's bag span stays <= 127, so window w covers bags
    [first_bag_w, first_bag_w+128).
  - indirect-DMA gather of each window's packed rows -> Gp [128, ncw*112].
  - DVE unpack of the 4+2+1 bit planes (fused shift+and into strided APs)
    -> biased 7-bit values; scalar-engine dequant via activation
    Copy(scale=DELTA, bias=-64*DELTA) -> Gbf (bf16).
  - one-hot masks built on DVE: mask[i, b] = (seg_local[i] == b) via
    is_equal against an iota row, seg_local broadcast along free dim.
  - PE matmul psum[bag, d] += mask_j.T @ Gbf_j accumulated over the
    window's chunks in PSUM, then copied (bf16) to SBUF and stored to DRAM
    rows [w*128, (w+1)*128).
  - Host adds overlapping window blocks into the final [B, D] per table
    (consecutive windows share at most the boundary bag).

NOTE: multi-column idx APs misaddress on HW (verified in an earlier
session) — the generic indirect DMA honors one index per partition, so
gathers stay per-chunk.
"""

import os
import sys

sys.path.insert(0, "/opt/trn_rl_repo")

import ml_dtypes
import numpy as np

import jax

# Persistent compilation cache: run_bass_via_pjrt builds a fresh jit closure
# per call, so without this every execute pays ~0.6s of re-compile; with it
# the lowered executable is keyed by content and reloaded from disk.
jax.config.update("jax_compilation_cache_dir", "/tmp/jax_emb_cache")
jax.config.update("jax_persistent_cache_min_entry_size_bytes", -1)
jax.config.update("jax_persistent_cache_min_compile_time_secs", 0)

import concourse.bacc as bacc
import concourse.bass as bass
import concourse.mybir as mybir
import concourse.tile as tile
from concourse.bass_utils import run_bass_kernel_spmd

T_TABLES = 8
N_ROWS = 200000
D = 128
B_BAGS = 8192
L_IDX = 163840
P = 128
NCHUNKS = L_IDX // P  # 1280
PB = 112  # packed bytes per row: 64 (4-bit plane) + 32 (2-bit) + 16 (1-bit)

TRACE = os.environ.get("EMB_TRACE", "0") == "1"
MAX_CPW = int(os.environ.get("EMB_MAX_CPW", "20"))

LAST_EXEC_NS = None
LAST_RESULTS = None
LAST_NC = None
LAST_INMAPS = None


def _build_program(cpw: int, windows: list[tuple[int, int]], nu: int, delta: float):
    """Build the SPMD Bass program. windows = [(chunk_lo, chunk_hi), ...]."""
    nc = bacc.Bacc(None, target_bir_lowering=False)
    w_d = nc.dram_tensor("wp", [nu, PB], mybir.dt.int8, kind="ExternalInput")
    # gidx packs row index (bits 0-17) and local bag id (bits 18-24)
    gidx_d = nc.dram_tensor("gidx", [P, NCHUNKS], mybir.dt.int32, kind="ExternalInput")
    W = len(windows)
    AO = mybir.AluOpType
    # out rows: [0,B) final bags (scatter targets), [B, B+W) boundary
    # partials, [B+W, B+2W) per-window dump rows (no write overlap anywhere)
    outrows = B_BAGS + 2 * W
    obag_d = nc.dram_tensor("obag", [P, W], mybir.dt.int32, kind="ExternalInput")
    out_d = nc.dram_tensor("out", [outrows, D], mybir.dt.bfloat16, kind="ExternalOutput")

    with tile.TileContext(nc) as tc:
        with (
            tc.tile_pool(name="const", bufs=1) as cpool,
            tc.tile_pool(name="g", bufs=3) as gpool,
            tc.tile_pool(name="m", bufs=3) as mpool,
            tc.tile_pool(name="st", bufs=4) as spool,
            tc.tile_pool(name="ps", bufs=4, space="PSUM") as ppool,
        ):
            gp_sb = cpool.tile([P, NCHUNKS], mybir.dt.int32)
            idx_sb = cpool.tile([P, NCHUNKS], mybir.dt.int32)
            seg32_sb = cpool.tile([P, NCHUNKS], mybir.dt.int32)
            seg_sb = cpool.tile([P, NCHUNKS], mybir.dt.bfloat16)
            iota32_sb = cpool.tile([P, P], mybir.dt.int32)
            iota_sb = cpool.tile([P, P], mybir.dt.bfloat16)
            obag_sb = cpool.tile([P, W], mybir.dt.int32)
            nc.sync.dma_start(out=obag_sb[:], in_=obag_d[:])
            nc.sync.dma_start(out=gp_sb[:], in_=gidx_d[:])
            nc.vector.tensor_scalar(
                out=idx_sb[:], in0=gp_sb[:], scalar1=0x3FFFF, scalar2=None,
                op0=mybir.AluOpType.bitwise_and,
            )
            nc.vector.tensor_scalar(
                out=seg32_sb[:], in0=gp_sb[:], scalar1=18, scalar2=None,
                op0=mybir.AluOpType.logical_shift_right,
            )
            nc.vector.tensor_copy(seg_sb[:], seg32_sb[:])
            nc.gpsimd.iota(iota32_sb[:], pattern=[[1, P]], base=0, channel_multiplier=0)
            nc.vector.tensor_copy(iota_sb[:], iota32_sb[:])

            def strided(tile_, start, gstride, n, istride, inner):
                """3-level AP on an SBUF int8 tile: n groups at gstride from
                element offset start, inner elems at istride."""
                a = tile_[:]
                return bass.AP(
                    a.tensor, a.offset + start,
                    [list(a.ap[0]), [gstride, n], [istride, inner]],
                )

            for w, (lo, hi) in enumerate(windows):
                ncw = hi - lo
                gp8 = gpool.tile([P, cpw * PB], mybir.dt.int8, tag="gp8")
                for j in range(ncw):
                    nc.gpsimd.indirect_dma_start(
                        out=gp8[:, j * PB : (j + 1) * PB],
                        out_offset=None,
                        in_=w_d[:],
                        in_offset=bass.IndirectOffsetOnAxis(
                            ap=idx_sb[:, lo + j : lo + j + 1], axis=0
                        ),
                    )
                # unpack 4+2+1 bit planes -> acc[k] = ((p4<<3)|(p2<<1)|p1)
                acc = gpool.tile([P, cpw * D], mybir.dt.int8, tag="acc")
                tmp = gpool.tile([P, cpw * D], mybir.dt.int8, tag="tmp")
                nc.vector.tensor_scalar(
                    out=strided(acc, 0, D, ncw, 2, 64),
                    in0=strided(gp8, 0, PB, ncw, 1, 64),
                    scalar1=1, scalar2=0x78,
                    op0=AO.logical_shift_right, op1=AO.bitwise_and)
                nc.vector.tensor_scalar(
                    out=strided(acc, 1, D, ncw, 2, 64),
                    in0=strided(gp8, 0, PB, ncw, 1, 64),
                    scalar1=3, scalar2=0x78,
                    op0=AO.logical_shift_left, op1=AO.bitwise_and)
                for s in range(4):
                    sh = 5 - 2 * s
                    op = AO.logical_shift_right if sh >= 0 else AO.logical_shift_left
                    nc.vector.tensor_scalar(
                        out=strided(tmp, s, D, ncw, 4, 32),
                        in0=strided(gp8, 64, PB, ncw, 1, 32),
                        scalar1=abs(sh), scalar2=6, op0=op, op1=AO.bitwise_and)
                nc.vector.tensor_tensor(
                    out=acc[:, : ncw * D], in0=acc[:, : ncw * D],
                    in1=tmp[:, : ncw * D], op=AO.bitwise_or)
                for s in range(8):
                    nc.vector.tensor_scalar(
                        out=strided(tmp, s, D, ncw, 8, 16),
                        in0=strided(gp8, 96, PB, ncw, 1, 16),
                        scalar1=7 - s, scalar2=1,
                        op0=AO.logical_shift_right, op1=AO.bitwise_and)
                nc.vector.tensor_tensor(
                    out=acc[:, : ncw * D], in0=acc[:, : ncw * D],
                    in1=tmp[:, : ncw * D], op=AO.bitwise_or)
                gbf_sb = gpool.tile([P, cpw * D], mybir.dt.bfloat16, tag="gbf")
                nc.scalar.activation(
                    gbf_sb[:, : ncw * D], acc[:, : ncw * D],
                    mybir.ActivationFunctionType.Copy,
                    bias=-64.0 * delta, scale=delta)
                mask_sb = mpool.tile([P, cpw * P], mybir.dt.bfloat16, tag="m")
                for j in range(ncw):
                    nc.vector.tensor_tensor(
                        out=mask_sb[:, j * P : (j + 1) * P],
                        in0=seg_sb[:, lo + j : lo + j + 1].to_broadcast([P, P]),
                        in1=iota_sb[:],
                        op=mybir.AluOpType.is_equal,
                    )
                psum = ppool.tile([P, D], mybir.dt.float32)
                for j in range(ncw):
                    nc.tensor.matmul(
                        out=psum[:],
                        lhsT=mask_sb[:, j * P : (j + 1) * P],
                        rhs=gbf_sb[:, j * D : (j + 1) * D],
                        start=(j == 0),
                        stop=(j == ncw - 1),
                    )
                stage = spool.tile([P, D], mybir.dt.bfloat16, tag="st")
                nc.scalar.copy(out=stage[:], in_=psum[:])
                nc.gpsimd.indirect_dma_start(
                    out=out_d[:],
                    out_offset=bass.IndirectOffsetOnAxis(
                        ap=obag_sb[:, w : w + 1], axis=0
                    ),
                    in_=stage[:],
                    in_offset=None,
                )

            # Consume the out-store DMAs so the tail drain stays under the
            # TPB_CTRL sync-wait limit: one readback touching the tensor.
            scrap = cpool.tile([P, 1], mybir.dt.bfloat16)
            nc.sync.dma_start(out=scrap[:, :], in_=out_d[0:P, 0:1])
    nc.finalize()
    return nc


def kernel(weights, values, offsets):
    global LAST_EXEC_NS, LAST_RESULTS
    weights = np.ascontiguousarray(np.asarray(weights), dtype=np.float32)
    values = np.asarray(values)
    offsets = np.asarray(offsets)
    vals32 = values.astype(np.int32)
    offs = offsets.astype(np.int64)

    # per-table bag id for every index position
    seg = np.empty((T_TABLES, L_IDX), np.int64)
    ar = np.arange(L_IDX)
    for t in range(T_TABLES):
        seg[t] = np.searchsorted(offs[t, 1:], ar, side="right")

    # greedy variable-length windows: extend while every table's bag span
    # stays <= 127 (so one 128-row psum block covers the window's bags)
    windows = []
    lo = 0
    while lo < NCHUNKS:
        hi = lo + 1
        while hi < NCHUNKS and hi - lo < MAX_CPW:
            if (seg[:, (hi + 1) * P - 1] - seg[:, lo * P]).max() > 127:
                break
            hi += 1
        windows.append((lo, hi))
        lo = hi
    for lo, hi in windows:  # safety: masks only cover local bags 0..127
        assert (seg[:, hi * P - 1] - seg[:, lo * P]).max() <= 127, \
            "pathological offsets: single window spans >128 bags"
    cpw = max(hi - lo for lo, hi in windows)
    W = len(windows)

    # dedup each table to its referenced rows; remap indices
    uniqs, invs = [], []
    for t in range(T_TABLES):
        u, inv = np.unique(vals32[t], return_inverse=True)
        uniqs.append(u)
        invs.append(inv.astype(np.int32))
    NU = max(u.size for u in uniqs)

    # one global symmetric 7-bit scale, baked into the program
    delta = float(np.abs(weights).max()) / 63.0
    if delta == 0.0:
        delta = 1.0
    wp = np.zeros((T_TABLES, NU, PB), np.int8)
    for t in range(T_TABLES):
        wq = np.rint(weights[t][uniqs[t]] * (1.0 / delta))
        w7 = (np.clip(wq, -64, 63) + 64.0).astype(np.uint8)  # biased 0..127
        p4 = (w7 >> 3).astype(np.uint8)
        p2 = ((w7 >> 1) & 3).astype(np.uint8)
        p1 = (w7 & 1).astype(np.uint8)
        b4 = (p4[:, 0::2] << 4) | p4[:, 1::2]
        b2 = ((p2[:, 0::4] << 6) | (p2[:, 1::4] << 4)
              | (p2[:, 2::4] << 2) | p2[:, 3::4])
        b1 = np.zeros((w7.shape[0], 16), np.uint8)
        for s in range(8):
            b1 |= p1[:, s::8] << (7 - s)
        wp[t, : uniqs[t].size] = np.concatenate(
            [b4, b2, b1], axis=1).astype(np.int8)

    first_bag = np.empty((T_TABLES, W), np.int64)
    gidx = np.empty((T_TABLES, P, NCHUNKS), np.int32)
    obag = np.empty((T_TABLES, P, W), np.int32)
    rr = np.arange(P)
    for t in range(T_TABLES):
        fb = seg[t, [lo * P for lo, _ in windows]]
        first_bag[t] = fb
        fb_per_idx = np.repeat(fb, [(hi - lo) * P for lo, hi in windows])
        sl = seg[t] - fb_per_idx  # local bag id, 0..127
        assert sl.min() >= 0 and sl.max() <= 127
        packed = invs[t].astype(np.int64) | (sl.astype(np.int64) << 18)
        gidx[t] = packed.astype(np.int32).reshape(NCHUNKS, P).T
        # scatter targets: final bag rows, boundary partial slot, dump row
        for w in range(W):
            bl = int(fb[w + 1] - fb[w]) if w + 1 < W else P
            col = np.where(
                (rr < bl) & (fb[w] + rr < B_BAGS), fb[w] + rr,
                np.where(rr == bl, B_BAGS + w, B_BAGS + W + w),
            )
            obag[t, :, w] = col

    global LAST_NC, LAST_INMAPS
    nc = _build_program(cpw, windows, NU, delta)
    in_maps = [
        {
            "wp": wp[t],
            "gidx": np.ascontiguousarray(gidx[t]),
            "obag": np.ascontiguousarray(obag[t]),
        }
        for t in range(T_TABLES)
    ]
    LAST_NC, LAST_INMAPS = nc, in_maps
    import time as _time

    t0 = _time.time()
    res = run_bass_kernel_spmd(
        nc, in_maps, core_ids=list(range(T_TABLES)), trace=TRACE
    )
    first_s = _time.time() - t0
    LAST_EXEC_NS = res.exec_time_ns
    LAST_RESULTS = res
    if LAST_EXEC_NS is None and os.environ.get("EMB_TIME_RERUN", "1") == "1":
        # no NTFF hook in this container: re-execute the cached executable;
        # wall time upper-bounds kernel time (still includes input transfer).
        t0 = _time.time()
        res = run_bass_kernel_spmd(nc, in_maps, core_ids=list(range(T_TABLES)))
        LAST_EXEC_NS = int((_time.time() - t0) * 1e9)
        print(f"[kernel] first call {first_s:.1f}s, cached re-exec "
              f"{LAST_EXEC_NS/1e6:.1f}ms (incl. host<->device transfer)")

    big = np.empty((T_TABLES, B_BAGS, D), np.float32)
    for t in range(T_TABLES):
        out_t = res.results[t]["out"].astype(np.float32)
        big[t] = out_t[:B_BAGS]
        for w in range(W - 1):  # fold boundary partials into their bags
            big[t, int(first_bag[t, w + 1])] += out_t[B_BAGS + w]
    return big.transpose(1, 0, 2).reshape(B_BAGS, T_TABLES * D)


# revision 27
# speedup vs baseline: 1.0993x; 1.0993x over previous
"""GroupedEmbeddingBag Trainium2 kernel.

Problem: T=8 tables of [N=200000, D=128] f32, per table L=163840 indices
pooled (sum) into B=8192 bags via CSR offsets. Output [B, T*D].

Sharding: table-wise — core t owns table t end-to-end (gather + pool).

The end-to-end metric here is dominated by host<->device transfer over the
axon tunnel (~35-75 MB/s), so the kernel minimizes wire bytes:
  - host dedups each table to its referenced rows (~112k of 200k) and
    remaps indices;
  - the deduped table ships 7-bit quantized (symmetric, one global
    compile-time scale DELTA; rel-err budget is 2e-2, 7-bit costs ~8e-3),
    plane-packed as 4+2+1 bit planes into 112 bytes/row and unpacked on
    the DVE with fused shift+and ops;
  - the local bag id rides in bits 18-24 of the index tensor; iota is
    generated on device;
  - output ships as bf16.

Device algorithm per core:
  - Host lays out the L indices as [128, 1280] "chunk" columns
    (chunk c = index positions [128c, 128c+128), lane p = position 128c+p).
  - Greedy variable-length windows of consecutive chunks, extended while
    every table's bag span stays <= 127, so window w covers bags
    [first_bag_w, first_bag_w+128).
  - indirect-DMA gather of each window's packed rows -> Gp [128, ncw*112].
  - DVE unpack of the 4+2+1 bit planes (fused shift+and into strided APs)
    -> biased 7-bit values; scalar-engine dequant via activation
    Copy(scale=DELTA, bias=-64*DELTA) -> Gbf (bf16).
  - one-hot masks built on DVE: mask[i, b] = (seg_local[i] == b) via
    is_equal against an iota row, seg_local broadcast along free dim.
  - PE matmul psum[bag, d] += mask_j.T @ Gbf_j accumulated over the
    window's chunks in PSUM, then copied (bf16) to SBUF and indirect-DMA
    scattered to DRAM: final bag rows [0,B) (disjoint across windows),
    boundary-partial slots [B,B+W), per-window dump rows [B+W,B+2W).
  - Host folds the W-1 boundary partials into their bags; everything else
    is already in final position.

NOTE: multi-column idx APs misaddress on HW (verified in an earlier
session) — the generic indirect DMA honors one index per partition, so
gathers stay per-chunk.
"""

import os
import sys

sys.path.insert(0, "/opt/trn_rl_repo")

import numpy as np

import jax

# Persistent compilation cache: run_bass_via_pjrt builds a fresh jit closure
# per call, so without this every execute pays ~0.6s of re-compile; with it
# the lowered executable is keyed by content and reloaded from disk.
jax.config.update("jax_compilation_cache_dir", "/tmp/jax_emb_cache")
jax.config.update("jax_persistent_cache_min_entry_size_bytes", -1)
jax.config.update("jax_persistent_cache_min_compile_time_secs", 0)

import concourse.bacc as bacc
import concourse.bass as bass
import concourse.mybir as mybir
import concourse.tile as tile
from concourse.bass_utils import run_bass_kernel_spmd

T_TABLES = 8
N_ROWS = 200000
D = 128
B_BAGS = 8192
L_IDX = 163840
P = 128
NCHUNKS = L_IDX // P  # 1280
PB = 112  # packed bytes per row: 64 (4-bit plane) + 32 (2-bit) + 16 (1-bit)

TRACE = os.environ.get("EMB_TRACE", "0") == "1"
MAX_CPW = int(os.environ.get("EMB_MAX_CPW", "20"))

LAST_EXEC_NS = None
LAST_RESULTS = None
LAST_NC = None
LAST_INMAPS = None


def _build_program(cpw: int, windows: list[tuple[int, int]], nu: int, delta: float):
    """Build the SPMD Bass program. windows = [(chunk_lo, chunk_hi), ...]."""
    nc = bacc.Bacc(None, target_bir_lowering=False)
    w_d = nc.dram_tensor("wp", [nu, PB], mybir.dt.int8, kind="ExternalInput")
    # gidx packs row index (bits 0-17) and local bag id (bits 18-24)
    gidx_d = nc.dram_tensor("gidx", [P, NCHUNKS], mybir.dt.int32, kind="ExternalInput")
    W = len(windows)
    AO = mybir.AluOpType
    # out rows: [0,B) final bags (scatter targets), [B, B+W) boundary
    # partials, [B+W, B+2W) per-window dump rows (no write overlap anywhere)
    outrows = B_BAGS + 2 * W
    obag_d = nc.dram_tensor("obag", [P, W], mybir.dt.int32, kind="ExternalInput")
    out_d = nc.dram_tensor("out", [outrows, D], mybir.dt.bfloat16, kind="ExternalOutput")

    with tile.TileContext(nc) as tc:
        with (
            tc.tile_pool(name="const", bufs=1) as cpool,
            tc.tile_pool(name="g", bufs=3) as gpool,
            tc.tile_pool(name="m", bufs=3) as mpool,
            tc.tile_pool(name="st", bufs=4) as spool,
            tc.tile_pool(name="ps", bufs=4, space="PSUM") as ppool,
        ):
            gp_sb = cpool.tile([P, NCHUNKS], mybir.dt.int32)
            idx_sb = cpool.tile([P, NCHUNKS], mybir.dt.int32)
            seg32_sb = cpool.tile([P, NCHUNKS], mybir.dt.int32)
            seg_sb = cpool.tile([P, NCHUNKS], mybir.dt.bfloat16)
            iota32_sb = cpool.tile([P, P], mybir.dt.int32)
            iota_sb = cpool.tile([P, P], mybir.dt.bfloat16)
            obag_sb = cpool.tile([P, W], mybir.dt.int32)
            nc.sync.dma_start(out=obag_sb[:], in_=obag_d[:])
            nc.sync.dma_start(out=gp_sb[:], in_=gidx_d[:])
            nc.vector.tensor_scalar(
                out=idx_sb[:], in0=gp_sb[:], scalar1=0x3FFFF, scalar2=None,
                op0=mybir.AluOpType.bitwise_and,
            )
            nc.vector.tensor_scalar(
                out=seg32_sb[:], in0=gp_sb[:], scalar1=18, scalar2=None,
                op0=mybir.AluOpType.logical_shift_right,
            )
            nc.vector.tensor_copy(seg_sb[:], seg32_sb[:])
            nc.gpsimd.iota(iota32_sb[:], pattern=[[1, P]], base=0, channel_multiplier=0)
            nc.vector.tensor_copy(iota_sb[:], iota32_sb[:])

            def strided(tile_, start, gstride, n, istride, inner):
                """3-level AP on an SBUF int8 tile: n groups at gstride from
                element offset start, inner elems at istride."""
                a = tile_[:]
                return bass.AP(
                    a.tensor, a.offset + start,
                    [list(a.ap[0]), [gstride, n], [istride, inner]],
                )

            for w, (lo, hi) in enumerate(windows):
                ncw = hi - lo
                gp8 = gpool.tile([P, cpw * PB], mybir.dt.int8, tag="gp8")
                for j in range(ncw):
                    nc.gpsimd.indirect_dma_start(
                        out=gp8[:, j * PB : (j + 1) * PB],
                        out_offset=None,
                        in_=w_d[:],
                        in_offset=bass.IndirectOffsetOnAxis(
                            ap=idx_sb[:, lo + j : lo + j + 1], axis=0
                        ),
                    )
                # unpack 4+2+1 bit planes -> acc[k] = ((p4<<3)|(p2<<1)|p1)
                acc = gpool.tile([P, cpw * D], mybir.dt.int8, tag="acc")
                tmp = gpool.tile([P, cpw * D], mybir.dt.int8, tag="tmp")
                nc.vector.tensor_scalar(
                    out=strided(acc, 0, D, ncw, 2, 64),
                    in0=strided(gp8, 0, PB, ncw, 1, 64),
                    scalar1=1, scalar2=0x78,
                    op0=AO.logical_shift_right, op1=AO.bitwise_and)
                nc.vector.tensor_scalar(
                    out=strided(acc, 1, D, ncw, 2, 64),
                    in0=strided(gp8, 0, PB, ncw, 1, 64),
                    scalar1=3, scalar2=0x78,
                    op0=AO.logical_shift_left, op1=AO.bitwise_and)
                for s in range(4):
                    sh = 5 - 2 * s
                    op = AO.logical_shift_right if sh >= 0 else AO.logical_shift_left
                    nc.vector.tensor_scalar(
                        out=strided(tmp, s, D, ncw, 4, 32),
                        in0=strided(gp8, 64, PB, ncw, 1, 32),
                        scalar1=abs(sh), scalar2=6, op0=op, op1=AO.bitwise_and)
                nc.vector.tensor_tensor(
                    out=acc[:, : ncw * D], in0=acc[:, : ncw * D],
                    in1=tmp[:, : ncw * D], op=AO.bitwise_or)
                for s in range(8):
                    nc.vector.tensor_scalar(
                        out=strided(tmp, s, D, ncw, 8, 16),
                        in0=strided(gp8, 96, PB, ncw, 1, 16),
                        scalar1=7 - s, scalar2=1,
                        op0=AO.logical_shift_right, op1=AO.bitwise_and)
                nc.vector.tensor_tensor(
                    out=acc[:, : ncw * D], in0=acc[:, : ncw * D],
                    in1=tmp[:, : ncw * D], op=AO.bitwise_or)
                gbf_sb = gpool.tile([P, cpw * D], mybir.dt.bfloat16, tag="gbf")
                nc.scalar.activation(
                    gbf_sb[:, : ncw * D], acc[:, : ncw * D],
                    mybir.ActivationFunctionType.Copy,
                    bias=-64.0 * delta, scale=delta)
                mask_sb = mpool.tile([P, cpw * P], mybir.dt.bfloat16, tag="m")
                for j in range(ncw):
                    nc.vector.tensor_tensor(
                        out=mask_sb[:, j * P : (j + 1) * P],
                        in0=seg_sb[:, lo + j : lo + j + 1].to_broadcast([P, P]),
                        in1=iota_sb[:],
                        op=mybir.AluOpType.is_equal,
                    )
                psum = ppool.tile([P, D], mybir.dt.float32)
                for j in range(ncw):
                    nc.tensor.matmul(
                        out=psum[:],
                        lhsT=mask_sb[:, j * P : (j + 1) * P],
                        rhs=gbf_sb[:, j * D : (j + 1) * D],
                        start=(j == 0),
                        stop=(j == ncw - 1),
                    )
                stage = spool.tile([P, D], mybir.dt.bfloat16, tag="st")
                nc.scalar.copy(out=stage[:], in_=psum[:])
                nc.gpsimd.indirect_dma_start(
                    out=out_d[:],
                    out_offset=bass.IndirectOffsetOnAxis(
                        ap=obag_sb[:, w : w + 1], axis=0
                    ),
                    in_=stage[:],
                    in_offset=None,
                )

            # Consume the out-store DMAs so the tail drain stays under the
            # TPB_CTRL sync-wait limit: one readback touching the tensor.
            scrap = cpool.tile([P, 1], mybir.dt.bfloat16)
            nc.sync.dma_start(out=scrap[:, :], in_=out_d[0:P, 0:1])
    nc.finalize()
    return nc


def kernel(weights, values, offsets):
    global LAST_EXEC_NS, LAST_RESULTS
    weights = np.ascontiguousarray(np.asarray(weights), dtype=np.float32)
    values = np.asarray(values)
    offsets = np.asarray(offsets)
    vals32 = values.astype(np.int32)
    offs = offsets.astype(np.int64)

    # per-table bag id for every index position
    seg = np.empty((T_TABLES, L_IDX), np.int64)
    ar = np.arange(L_IDX)
    for t in range(T_TABLES):
        seg[t] = np.searchsorted(offs[t, 1:], ar, side="right")

    # greedy variable-length windows: extend while every table's bag span
    # stays <= 127 (so one 128-row psum block covers the window's bags)
    windows = []
    lo = 0
    while lo < NCHUNKS:
        hi = lo + 1
        while hi < NCHUNKS and hi - lo < MAX_CPW:
            if (seg[:, (hi + 1) * P - 1] - seg[:, lo * P]).max() > 127:
                break
            hi += 1
        windows.append((lo, hi))
        lo = hi
    for lo, hi in windows:  # safety: masks only cover local bags 0..127
        assert (seg[:, hi * P - 1] - seg[:, lo * P]).max() <= 127, \
            "pathological offsets: single window spans >128 bags"
    cpw = max(hi - lo for lo, hi in windows)
    W = len(windows)

    # dedup each table to its referenced rows; remap indices
    uniqs, invs = [], []
    for t in range(T_TABLES):
        u, inv = np.unique(vals32[t], return_inverse=True)
        uniqs.append(u)
        invs.append(inv.astype(np.int32))
    NU = max(u.size for u in uniqs)

    # one global symmetric 7-bit scale, baked into the program
    delta = float(np.abs(weights).max()) / 63.0
    if delta == 0.0:
        delta = 1.0
    wp = np.zeros((T_TABLES, NU, PB), np.int8)
    for t in range(T_TABLES):
        wq = np.rint(weights[t][uniqs[t]] * (1.0 / delta))
        w7 = (np.clip(wq, -64, 63) + 64.0).astype(np.uint8)  # biased 0..127
        p4 = (w7 >> 3).astype(np.uint8)
        p2 = ((w7 >> 1) & 3).astype(np.uint8)
        p1 = (w7 & 1).astype(np.uint8)
        b4 = (p4[:, 0::2] << 4) | p4[:, 1::2]
        b2 = ((p2[:, 0::4] << 6) | (p2[:, 1::4] << 4)
              | (p2[:, 2::4] << 2) | p2[:, 3::4])
        b1 = np.zeros((w7.shape[0], 16), np.uint8)
        for s in range(8):
            b1 |= p1[:, s::8] << (7 - s)
        wp[t, : uniqs[t].size] = np.concatenate(
            [b4, b2, b1], axis=1).astype(np.int8)

    first_bag = np.empty((T_TABLES, W), np.int64)
    gidx = np.empty((T_TABLES, P, NCHUNKS), np.int32)
    obag = np.empty((T_TABLES, P, W), np.int32)
    rr = np.arange(P)
    for t in range(T_TABLES):
        fb = seg[t, [lo * P for lo, _ in windows]]
        first_bag[t] = fb
        fb_per_idx = np.repeat(fb, [(hi - lo) * P for lo, hi in windows])
        sl = seg[t] - fb_per_idx  # local bag id, 0..127
        assert sl.min() >= 0 and sl.max() <= 127
        packed = invs[t].astype(np.int64) | (sl.astype(np.int64) << 18)
        gidx[t] = packed.astype(np.int32).reshape(NCHUNKS, P).T
        # scatter targets: final bag rows, boundary partial slot, dump row
        for w in range(W):
            bl = int(fb[w + 1] - fb[w]) if w + 1 < W else P
            col = np.where(
                (rr < bl) & (fb[w] + rr < B_BAGS), fb[w] + rr,
                np.where(rr == bl, B_BAGS + w, B_BAGS + W + w),
            )
            obag[t, :, w] = col

    global LAST_NC, LAST_INMAPS
    nc = _build_program(cpw, windows, NU, delta)
    in_maps = [
        {
            "wp": wp[t],
            "gidx": np.ascontiguousarray(gidx[t]),
            "obag": np.ascontiguousarray(obag[t]),
        }
        for t in range(T_TABLES)
    ]
    LAST_NC, LAST_INMAPS = nc, in_maps
    import time as _time

    t0 = _time.time()
    res = run_bass_kernel_spmd(
        nc, in_maps, core_ids=list(range(T_TABLES)), trace=TRACE
    )
    first_s = _time.time() - t0
    LAST_EXEC_NS = res.exec_time_ns
    LAST_RESULTS = res
    if LAST_EXEC_NS is None and os.environ.get("EMB_TIME_RERUN", "1") == "1":
        # no NTFF hook in this container: re-execute the cached executable;
        # wall time upper-bounds kernel time (still includes input transfer).
        t0 = _time.time()
        res = run_bass_kernel_spmd(nc, in_maps, core_ids=list(range(T_TABLES)))
        LAST_EXEC_NS = int((_time.time() - t0) * 1e9)
        print(f"[kernel] first call {first_s:.1f}s, cached re-exec "
              f"{LAST_EXEC_NS/1e6:.1f}ms (incl. host<->device transfer)")

    big = np.empty((T_TABLES, B_BAGS, D), np.float32)
    for t in range(T_TABLES):
        out_t = res.results[t]["out"].astype(np.float32)
        big[t] = out_t[:B_BAGS]
        for w in range(W - 1):  # fold boundary partials into their bags
            big[t, int(first_bag[t, w + 1])] += out_t[B_BAGS + w]
    return big.transpose(1, 0, 2).reshape(B_BAGS, T_TABLES * D)


# revision 32
# speedup vs baseline: 1.2206x; 1.1103x over previous
"""GroupedEmbeddingBag Trainium2 kernel.

Problem: T=8 tables of [N=200000, D=128] f32, per table L=163840 indices
pooled (sum) into B=8192 bags via CSR offsets. Output [B, T*D].

Sharding: table-wise — core t owns table t end-to-end (gather + pool).

The end-to-end metric here is dominated by host<->device transfer over the
axon tunnel (~35-75 MB/s), so the kernel minimizes wire bytes:
  - host dedups each table to its referenced rows (~112k of 200k) and
    remaps indices;
  - the deduped table ships 5-bit quantized (symmetric, one global
    compile-time scale DELTA), plane-packed as 4+1 bit planes into 80
    bytes/row and unpacked on the DVE with fused shift+and ops. Naive
    5-bit rounding would cost ~3e-2 rel err (budget 2e-2), but the host
    quantizes with per-bag error feedback: rows referenced exactly once
    (~54% of references) are rounded Floyd-Steinberg style so each bag's
    running residual — including the plain-rounding errors of shared
    rows — telescopes to ~delta/2. Measured rel err ~7.9e-3, same class
    as naive 7-bit;
  - the local bag id rides in bits 18-24 of the index tensor; iota is
    generated on device;
  - output ships as bf16.

Device algorithm per core:
  - Host lays out the L indices as [128, 1280] "chunk" columns
    (chunk c = index positions [128c, 128c+128), lane p = position 128c+p).
  - Greedy variable-length windows of consecutive chunks, extended while
    every table's bag span stays <= 127, so window w covers bags
    [first_bag_w, first_bag_w+128).
  - indirect-DMA gather of each window's packed rows -> Gp [128, ncw*80].
  - DVE unpack of the 4+1 bit planes (fused shift+and into strided APs)
    -> biased 5-bit values; scalar-engine dequant via activation
    Copy(scale=DELTA, bias=-16*DELTA) -> Gbf (bf16).
  - one-hot masks built on DVE: mask[i, b] = (seg_local[i] == b) via
    is_equal against an iota row, seg_local broadcast along free dim.
  - PE matmul psum[bag, d] += mask_j.T @ Gbf_j accumulated over the
    window's chunks in PSUM, then copied (bf16) to SBUF and indirect-DMA
    scattered to DRAM: final bag rows [0,B) (disjoint across windows),
    boundary-partial slots [B,B+W), per-window dump rows [B+W,B+2W).
  - Host folds the W-1 boundary partials into their bags; everything else
    is already in final position.

NOTE: multi-column idx APs misaddress on HW (verified in an earlier
session) — the generic indirect DMA honors one index per partition, so
gathers stay per-chunk.
"""

import os
import sys

sys.path.insert(0, "/opt/trn_rl_repo")

import numpy as np

import jax

# Persistent compilation cache: run_bass_via_pjrt builds a fresh jit closure
# per call, so without this every execute pays ~0.6s of re-compile; with it
# the lowered executable is keyed by content and reloaded from disk.
jax.config.update("jax_compilation_cache_dir", "/tmp/jax_emb_cache")
jax.config.update("jax_persistent_cache_min_entry_size_bytes", -1)
jax.config.update("jax_persistent_cache_min_compile_time_secs", 0)

import concourse.bacc as bacc
import concourse.bass as bass
import concourse.mybir as mybir
import concourse.tile as tile
from concourse.bass_utils import run_bass_kernel_spmd

T_TABLES = 8
N_ROWS = 200000
D = 128
B_BAGS = 8192
L_IDX = 163840
P = 128
NCHUNKS = L_IDX // P  # 1280
PB = 80  # packed bytes per row: 64 (4-bit plane) + 16 (1-bit plane)

TRACE = os.environ.get("EMB_TRACE", "0") == "1"
MAX_CPW = int(os.environ.get("EMB_MAX_CPW", "20"))

LAST_EXEC_NS = None
LAST_RESULTS = None
LAST_NC = None
LAST_INMAPS = None


def _build_program(cpw: int, windows: list[tuple[int, int]], nu: int, delta: float):
    """Build the SPMD Bass program. windows = [(chunk_lo, chunk_hi), ...]."""
    nc = bacc.Bacc(None, target_bir_lowering=False)
    w_d = nc.dram_tensor("wp", [nu, PB], mybir.dt.int8, kind="ExternalInput")
    # gidx packs row index (bits 0-17) and local bag id (bits 18-24)
    gidx_d = nc.dram_tensor("gidx", [P, NCHUNKS], mybir.dt.int32, kind="ExternalInput")
    W = len(windows)
    AO = mybir.AluOpType
    # out rows: [0,B) final bags (scatter targets), [B, B+W) boundary
    # partials, [B+W, B+2W) per-window dump rows (no write overlap anywhere)
    outrows = B_BAGS + 2 * W
    obag_d = nc.dram_tensor("obag", [P, W], mybir.dt.int32, kind="ExternalInput")
    out_d = nc.dram_tensor("out", [outrows, D], mybir.dt.bfloat16, kind="ExternalOutput")

    with tile.TileContext(nc) as tc:
        with (
            tc.tile_pool(name="const", bufs=1) as cpool,
            tc.tile_pool(name="g", bufs=3) as gpool,
            tc.tile_pool(name="m", bufs=3) as mpool,
            tc.tile_pool(name="st", bufs=4) as spool,
            tc.tile_pool(name="ps", bufs=4, space="PSUM") as ppool,
        ):
            gp_sb = cpool.tile([P, NCHUNKS], mybir.dt.int32)
            idx_sb = cpool.tile([P, NCHUNKS], mybir.dt.int32)
            seg32_sb = cpool.tile([P, NCHUNKS], mybir.dt.int32)
            seg_sb = cpool.tile([P, NCHUNKS], mybir.dt.bfloat16)
            iota32_sb = cpool.tile([P, P], mybir.dt.int32)
            iota_sb = cpool.tile([P, P], mybir.dt.bfloat16)
            obag_sb = cpool.tile([P, W], mybir.dt.int32)
            nc.sync.dma_start(out=obag_sb[:], in_=obag_d[:])
            nc.sync.dma_start(out=gp_sb[:], in_=gidx_d[:])
            nc.vector.tensor_scalar(
                out=idx_sb[:], in0=gp_sb[:], scalar1=0x3FFFF, scalar2=None,
                op0=mybir.AluOpType.bitwise_and,
            )
            nc.vector.tensor_scalar(
                out=seg32_sb[:], in0=gp_sb[:], scalar1=18, scalar2=None,
                op0=mybir.AluOpType.logical_shift_right,
            )
            nc.vector.tensor_copy(seg_sb[:], seg32_sb[:])
            nc.gpsimd.iota(iota32_sb[:], pattern=[[1, P]], base=0, channel_multiplier=0)
            nc.vector.tensor_copy(iota_sb[:], iota32_sb[:])

            def strided(tile_, start, gstride, n, istride, inner):
                """3-level AP on an SBUF int8 tile: n groups at gstride from
                element offset start, inner elems at istride."""
                a = tile_[:]
                return bass.AP(
                    a.tensor, a.offset + start,
                    [list(a.ap[0]), [gstride, n], [istride, inner]],
                )

            for w, (lo, hi) in enumerate(windows):
                ncw = hi - lo
                gp8 = gpool.tile([P, cpw * PB], mybir.dt.int8, tag="gp8")
                for j in range(ncw):
                    nc.gpsimd.indirect_dma_start(
                        out=gp8[:, j * PB : (j + 1) * PB],
                        out_offset=None,
                        in_=w_d[:],
                        in_offset=bass.IndirectOffsetOnAxis(
                            ap=idx_sb[:, lo + j : lo + j + 1], axis=0
                        ),
                    )
                # unpack 4+1 bit planes -> acc[k] = ((p4<<1)|p1), biased 0..31
                acc = gpool.tile([P, cpw * D], mybir.dt.int8, tag="acc")
                tmp = gpool.tile([P, cpw * D], mybir.dt.int8, tag="tmp")
                nc.vector.tensor_scalar(
                    out=strided(acc, 0, D, ncw, 2, 64),
                    in0=strided(gp8, 0, PB, ncw, 1, 64),
                    scalar1=3, scalar2=0x1E,
                    op0=AO.logical_shift_right, op1=AO.bitwise_and)
                nc.vector.tensor_scalar(
                    out=strided(acc, 1, D, ncw, 2, 64),
                    in0=strided(gp8, 0, PB, ncw, 1, 64),
                    scalar1=1, scalar2=0x1E,
                    op0=AO.logical_shift_left, op1=AO.bitwise_and)
                for s in range(8):
                    nc.vector.tensor_scalar(
                        out=strided(tmp, s, D, ncw, 8, 16),
                        in0=strided(gp8, 64, PB, ncw, 1, 16),
                        scalar1=7 - s, scalar2=1,
                        op0=AO.logical_shift_right, op1=AO.bitwise_and)
                nc.vector.tensor_tensor(
                    out=acc[:, : ncw * D], in0=acc[:, : ncw * D],
                    in1=tmp[:, : ncw * D], op=AO.bitwise_or)
                gbf_sb = gpool.tile([P, cpw * D], mybir.dt.bfloat16, tag="gbf")
                nc.scalar.activation(
                    gbf_sb[:, : ncw * D], acc[:, : ncw * D],
                    mybir.ActivationFunctionType.Copy,
                    bias=-16.0 * delta, scale=delta)
                mask_sb = mpool.tile([P, cpw * P], mybir.dt.bfloat16, tag="m")
                for j in range(ncw):
                    nc.vector.tensor_tensor(
                        out=mask_sb[:, j * P : (j + 1) * P],
                        in0=seg_sb[:, lo + j : lo + j + 1].to_broadcast([P, P]),
                        in1=iota_sb[:],
                        op=mybir.AluOpType.is_equal,
                    )
                psum = ppool.tile([P, D], mybir.dt.float32)
                for j in range(ncw):
                    nc.tensor.matmul(
                        out=psum[:],
                        lhsT=mask_sb[:, j * P : (j + 1) * P],
                        rhs=gbf_sb[:, j * D : (j + 1) * D],
                        start=(j == 0),
                        stop=(j == ncw - 1),
                    )
                stage = spool.tile([P, D], mybir.dt.bfloat16, tag="st")
                nc.scalar.copy(out=stage[:], in_=psum[:])
                nc.gpsimd.indirect_dma_start(
                    out=out_d[:],
                    out_offset=bass.IndirectOffsetOnAxis(
                        ap=obag_sb[:, w : w + 1], axis=0
                    ),
                    in_=stage[:],
                    in_offset=None,
                )

            # Consume the out-store DMAs so the tail drain stays under the
            # TPB_CTRL sync-wait limit: one readback touching the tensor.
            scrap = cpool.tile([P, 1], mybir.dt.bfloat16)
            nc.sync.dma_start(out=scrap[:, :], in_=out_d[0:P, 0:1])
    nc.finalize()
    return nc


def kernel(weights, values, offsets):
    global LAST_EXEC_NS, LAST_RESULTS
    weights = np.ascontiguousarray(np.asarray(weights), dtype=np.float32)
    values = np.asarray(values)
    offsets = np.asarray(offsets)
    vals32 = values.astype(np.int32)
    offs = offsets.astype(np.int64)

    # per-table bag id for every index position
    seg = np.empty((T_TABLES, L_IDX), np.int64)
    ar = np.arange(L_IDX)
    for t in range(T_TABLES):
        seg[t] = np.searchsorted(offs[t, 1:], ar, side="right")

    # greedy variable-length windows: extend while every table's bag span
    # stays <= 127 (so one 128-row psum block covers the window's bags)
    windows = []
    lo = 0
    while lo < NCHUNKS:
        hi = lo + 1
        while hi < NCHUNKS and hi - lo < MAX_CPW:
            if (seg[:, (hi + 1) * P - 1] - seg[:, lo * P]).max() > 127:
                break
            hi += 1
        windows.append((lo, hi))
        lo = hi
    for lo, hi in windows:  # safety: masks only cover local bags 0..127
        assert (seg[:, hi * P - 1] - seg[:, lo * P]).max() <= 127, \
            "pathological offsets: single window spans >128 bags"
    cpw = max(hi - lo for lo, hi in windows)
    W = len(windows)

    # dedup each table to its referenced rows; remap indices
    uniqs, invs = [], []
    for t in range(T_TABLES):
        u, inv = np.unique(vals32[t], return_inverse=True)
        uniqs.append(u)
        invs.append(inv.astype(np.int32))
    NU = max(u.size for u in uniqs)

    # one global symmetric 5-bit scale, baked into the program. Naive 5-bit
    # rounding is too lossy; instead each bag's exclusively-referenced rows
    # are rounded Floyd-Steinberg style, absorbing the bag's running
    # residual (including the plain-rounding errors of shared rows), so
    # per-bag quantization error telescopes to ~delta/2.
    delta = float(np.abs(weights).max()) / 15.0
    if delta == 0.0:
        delta = 1.0
    wp = np.zeros((T_TABLES, NU, PB), np.int8)
    for t in range(T_TABLES):
        nu = uniqs[t].size
        inv = invs[t]
        x = weights[t][uniqs[t]] * (1.0 / delta)          # [nu, D] ideal
        refcount = np.bincount(inv, minlength=nu)
        excl = refcount == 1
        w5 = np.clip(np.rint(x), -16, 15)                 # shared rows: naive
        e = x - w5
        carry = np.zeros((B_BAGS, D))
        sh_pos = np.nonzero(~excl[inv])[0]
        np.add.at(carry, seg[t][sh_pos], e[inv[sh_pos]])
        ex_pos = np.nonzero(excl[inv])[0]                 # bag-sorted order
        ex_rows = inv[ex_pos]
        ex_bags = seg[t][ex_pos]
        counts = np.bincount(ex_bags, minlength=B_BAGS)
        starts = np.concatenate([[0], np.cumsum(counts)[:-1]])
        resid = carry
        for j in range(int(counts.max())):                # step j of every bag
            act = np.nonzero(counts > j)[0]
            rows_j = ex_rows[starts[act] + j]
            tv = x[rows_j] + resid[act]
            q = np.clip(np.rint(tv), -16, 15)
            w5[rows_j] = q
            resid[act] = tv - q
        v = (w5 + 16.0).astype(np.uint8)                  # biased 0..31
        p4 = (v >> 1).astype(np.uint8)
        p1 = (v & 1).astype(np.uint8)
        b4 = (p4[:, 0::2] << 4) | p4[:, 1::2]
        b1 = np.zeros((nu, 16), np.uint8)
        for s in range(8):
            b1 |= p1[:, s::8] << (7 - s)
        wp[t, :nu] = np.concatenate([b4, b1], axis=1).astype(np.int8)

    first_bag = np.empty((T_TABLES, W), np.int64)
    gidx = np.empty((T_TABLES, P, NCHUNKS), np.int32)
    obag = np.empty((T_TABLES, P, W), np.int32)
    rr = np.arange(P)
    for t in range(T_TABLES):
        fb = seg[t, [lo * P for lo, _ in windows]]
        first_bag[t] = fb
        fb_per_idx = np.repeat(fb, [(hi - lo) * P for lo, hi in windows])
        sl = seg[t] - fb_per_idx  # local bag id, 0..127
        assert sl.min() >= 0 and sl.max() <= 127
        packed = invs[t].astype(np.int64) | (sl.astype(np.int64) << 18)
        gidx[t] = packed.astype(np.int32).reshape(NCHUNKS, P).T
        # scatter targets: final bag rows, boundary partial slot, dump row
        for w in range(W):
            bl = int(fb[w + 1] - fb[w]) if w + 1 < W else P
            col = np.where(
                (rr < bl) & (fb[w] + rr < B_BAGS), fb[w] + rr,
                np.where(rr == bl, B_BAGS + w, B_BAGS + W + w),
            )
            obag[t, :, w] = col

    global LAST_NC, LAST_INMAPS
    nc = _build_program(cpw, windows, NU, delta)
    in_maps = [
        {
            "wp": wp[t],
            "gidx": np.ascontiguousarray(gidx[t]),
            "obag": np.ascontiguousarray(obag[t]),
        }
        for t in range(T_TABLES)
    ]
    LAST_NC, LAST_INMAPS = nc, in_maps
    import time as _time

    t0 = _time.time()
    res = run_bass_kernel_spmd(
        nc, in_maps, core_ids=list(range(T_TABLES)), trace=TRACE
    )
    first_s = _time.time() - t0
    LAST_EXEC_NS = res.exec_time_ns
    LAST_RESULTS = res
    if LAST_EXEC_NS is None and os.environ.get("EMB_TIME_RERUN", "1") == "1":
        # no NTFF hook in this container: re-execute the cached executable;
        # wall time upper-bounds kernel time (still includes input transfer).
        t0 = _time.time()
        res = run_bass_kernel_spmd(nc, in_maps, core_ids=list(range(T_TABLES)))
        LAST_EXEC_NS = int((_time.time() - t0) * 1e9)
        print(f"[kernel] first call {first_s:.1f}s, cached re-exec "
              f"{LAST_EXEC_NS/1e6:.1f}ms (incl. host<->device transfer)")

    big = np.empty((T_TABLES, B_BAGS, D), np.float32)
    for t in range(T_TABLES):
        out_t = res.results[t]["out"].astype(np.float32)
        big[t] = out_t[:B_BAGS]
        for w in range(W - 1):  # fold boundary partials into their bags
            big[t, int(first_bag[t, w + 1])] += out_t[B_BAGS + w]
    return big.transpose(1, 0, 2).reshape(B_BAGS, T_TABLES * D)


# revision 37
# speedup vs baseline: 1.4151x; 1.1594x over previous
"""GroupedEmbeddingBag Trainium2 kernel.

Problem: T=8 tables of [N=200000, D=128] f32, per table L=163840 indices
pooled (sum) into B=8192 bags via CSR offsets. Output [B, T*D].

Sharding: table-wise — core t owns table t end-to-end (gather + pool).

The end-to-end metric here is dominated by host<->device transfer over the
axon tunnel (~35-75 MB/s), so the kernel minimizes wire bytes:
  - host dedups each table to its referenced rows (~112k of 200k) and
    remaps indices;
  - the deduped table ships 5-bit quantized (symmetric, one global
    compile-time scale DELTA), plane-packed as 4+1 bit planes into 80
    bytes/row and unpacked on the DVE with fused shift+and ops. Naive
    5-bit rounding would cost ~3e-2 rel err (budget 2e-2), but the host
    quantizes with per-bag error feedback: rows referenced exactly once
    (~54% of references) are rounded Floyd-Steinberg style so each bag's
    running residual — including the plain-rounding errors of shared
    rows — telescopes to ~delta/2. Measured rel err ~7.9e-3, same class
    as naive 7-bit;
  - the local bag id rides in bits 17-23 of the 24-bit packed index
    entries, shipped as three uint8 planes; iota is generated on device;
  - output ships as bf16.

Device algorithm per core:
  - Host lays out the L indices as [128, 1280] "chunk" columns
    (chunk c = index positions [128c, 128c+128), lane p = position 128c+p).
  - Greedy variable-length windows of consecutive chunks, extended while
    every table's bag span stays <= 127, so window w covers bags
    [first_bag_w, first_bag_w+128).
  - indirect-DMA gather of each window's packed rows -> Gp [128, ncw*80].
  - DVE unpack of the 4+1 bit planes (fused shift+and into strided APs)
    -> biased 5-bit values; scalar-engine dequant via activation
    Copy(scale=DELTA, bias=-16*DELTA) -> Gbf (bf16).
  - one-hot masks built on DVE: mask[i, b] = (seg_local[i] == b) via
    is_equal against an iota row, seg_local broadcast along free dim.
  - PE matmul psum[bag, d] += mask_j.T @ Gbf_j accumulated over the
    window's chunks in PSUM, then copied (bf16) to SBUF and indirect-DMA
    scattered to DRAM: final bag rows [0,B) (disjoint across windows),
    boundary-partial slots [B,B+W), per-window dump rows [B+W,B+2W).
  - Host folds the W-1 boundary partials into their bags; everything else
    is already in final position.

NOTE: multi-column idx APs misaddress on HW (verified in an earlier
session) — the generic indirect DMA honors one index per partition, so
gathers stay per-chunk.
"""

import os
import sys

sys.path.insert(0, "/opt/trn_rl_repo")

import numpy as np

import jax

# Persistent compilation cache: run_bass_via_pjrt builds a fresh jit closure
# per call, so without this every execute pays ~0.6s of re-compile; with it
# the lowered executable is keyed by content and reloaded from disk.
jax.config.update("jax_compilation_cache_dir", "/tmp/jax_emb_cache")
jax.config.update("jax_persistent_cache_min_entry_size_bytes", -1)
jax.config.update("jax_persistent_cache_min_compile_time_secs", 0)

import concourse.bacc as bacc
import concourse.bass as bass
import concourse.mybir as mybir
import concourse.tile as tile
from concourse.bass_utils import run_bass_kernel_spmd

T_TABLES = 8
N_ROWS = 200000
D = 128
B_BAGS = 8192
L_IDX = 163840
P = 128
NCHUNKS = L_IDX // P  # 1280
PB = 80  # packed bytes per row: 64 (4-bit plane) + 16 (1-bit plane)

TRACE = os.environ.get("EMB_TRACE", "0") == "1"
MAX_CPW = int(os.environ.get("EMB_MAX_CPW", "20"))

LAST_EXEC_NS = None
LAST_RESULTS = None
LAST_NC = None
LAST_INMAPS = None


def _build_program(cpw: int, windows: list[tuple[int, int]], nu: int, delta: float):
    """Build the SPMD Bass program. windows = [(chunk_lo, chunk_hi), ...]."""
    nc = bacc.Bacc(None, target_bir_lowering=False)
    w_d = nc.dram_tensor("wp", [nu, PB], mybir.dt.int8, kind="ExternalInput")
    # gidx packs row index (bits 0-16) and local bag id (bits 17-23),
    # shipped as three uint8 planes (lo/mid/hi) to save a byte per entry
    gidx_d = nc.dram_tensor(
        "gidx", [P, 3 * NCHUNKS], mybir.dt.uint8, kind="ExternalInput")
    W = len(windows)
    AO = mybir.AluOpType
    # out rows: [0,B) final bags (scatter targets), [B, B+W) boundary
    # partials, row B+W the shared dump row (garbage tolerated, dropped)
    outrows = B_BAGS + W + 1
    obag_d = nc.dram_tensor("obag", [P, W], mybir.dt.int32, kind="ExternalInput")
    out_d = nc.dram_tensor("out", [outrows, D], mybir.dt.bfloat16, kind="ExternalOutput")

    with tile.TileContext(nc) as tc:
        with (
            tc.tile_pool(name="const", bufs=1) as cpool,
            tc.tile_pool(name="g", bufs=3) as gpool,
            tc.tile_pool(name="m", bufs=3) as mpool,
            tc.tile_pool(name="st", bufs=4) as spool,
            tc.tile_pool(name="ps", bufs=4, space="PSUM") as ppool,
        ):
            g8_sb = cpool.tile([P, 3 * NCHUNKS], mybir.dt.uint8)
            gp_sb = cpool.tile([P, NCHUNKS], mybir.dt.int32)
            u1_sb = cpool.tile([P, NCHUNKS], mybir.dt.int32)
            u2_sb = cpool.tile([P, NCHUNKS], mybir.dt.int32)
            idx_sb = cpool.tile([P, NCHUNKS], mybir.dt.int32)
            seg32_sb = cpool.tile([P, NCHUNKS], mybir.dt.int32)
            seg_sb = cpool.tile([P, NCHUNKS], mybir.dt.bfloat16)
            iota32_sb = cpool.tile([P, P], mybir.dt.int32)
            iota_sb = cpool.tile([P, P], mybir.dt.bfloat16)
            obag_sb = cpool.tile([P, W], mybir.dt.int32)
            nc.sync.dma_start(out=obag_sb[:], in_=obag_d[:])
            nc.sync.dma_start(out=g8_sb[:], in_=gidx_d[:])
            # reconstruct 24-bit packed entries: copies zero-extend uint8 ->
            # int32, shifts must run at int32 width (ALU ops run at input dtype)
            nc.vector.tensor_copy(gp_sb[:], g8_sb[:, 0:NCHUNKS])
            nc.vector.tensor_copy(u1_sb[:], g8_sb[:, NCHUNKS : 2 * NCHUNKS])
            nc.vector.tensor_copy(u2_sb[:], g8_sb[:, 2 * NCHUNKS : 3 * NCHUNKS])
            nc.vector.tensor_scalar(
                out=u1_sb[:], in0=u1_sb[:], scalar1=8, scalar2=None,
                op0=AO.logical_shift_left)
            nc.vector.tensor_scalar(
                out=u2_sb[:], in0=u2_sb[:], scalar1=16, scalar2=None,
                op0=AO.logical_shift_left)
            nc.vector.tensor_tensor(
                out=gp_sb[:], in0=gp_sb[:], in1=u1_sb[:], op=AO.bitwise_or)
            nc.vector.tensor_tensor(
                out=gp_sb[:], in0=gp_sb[:], in1=u2_sb[:], op=AO.bitwise_or)
            nc.vector.tensor_scalar(
                out=idx_sb[:], in0=gp_sb[:], scalar1=0x1FFFF, scalar2=None,
                op0=AO.bitwise_and,
            )
            nc.vector.tensor_scalar(
                out=seg32_sb[:], in0=gp_sb[:], scalar1=17, scalar2=None,
                op0=AO.logical_shift_right,
            )
            nc.vector.tensor_copy(seg_sb[:], seg32_sb[:])
            nc.gpsimd.iota(iota32_sb[:], pattern=[[1, P]], base=0, channel_multiplier=0)
            nc.vector.tensor_copy(iota_sb[:], iota32_sb[:])

            def strided(tile_, start, gstride, n, istride, inner):
                """3-level AP on an SBUF int8 tile: n groups at gstride from
                element offset start, inner elems at istride."""
                a = tile_[:]
                return bass.AP(
                    a.tensor, a.offset + start,
                    [list(a.ap[0]), [gstride, n], [istride, inner]],
                )

            for w, (lo, hi) in enumerate(windows):
                ncw = hi - lo
                gp8 = gpool.tile([P, cpw * PB], mybir.dt.int8, tag="gp8")
                for j in range(ncw):
                    nc.gpsimd.indirect_dma_start(
                        out=gp8[:, j * PB : (j + 1) * PB],
                        out_offset=None,
                        in_=w_d[:],
                        in_offset=bass.IndirectOffsetOnAxis(
                            ap=idx_sb[:, lo + j : lo + j + 1], axis=0
                        ),
                    )
                # unpack 4+1 bit planes -> acc[k] = ((p4<<1)|p1), biased 0..31
                acc = gpool.tile([P, cpw * D], mybir.dt.int8, tag="acc")
                tmp = gpool.tile([P, cpw * D], mybir.dt.int8, tag="tmp")
                nc.vector.tensor_scalar(
                    out=strided(acc, 0, D, ncw, 2, 64),
                    in0=strided(gp8, 0, PB, ncw, 1, 64),
                    scalar1=3, scalar2=0x1E,
                    op0=AO.logical_shift_right, op1=AO.bitwise_and)
                nc.vector.tensor_scalar(
                    out=strided(acc, 1, D, ncw, 2, 64),
                    in0=strided(gp8, 0, PB, ncw, 1, 64),
                    scalar1=1, scalar2=0x1E,
                    op0=AO.logical_shift_left, op1=AO.bitwise_and)
                for s in range(8):
                    nc.vector.tensor_scalar(
                        out=strided(tmp, s, D, ncw, 8, 16),
                        in0=strided(gp8, 64, PB, ncw, 1, 16),
                        scalar1=7 - s, scalar2=1,
                        op0=AO.logical_shift_right, op1=AO.bitwise_and)
                nc.vector.tensor_tensor(
                    out=acc[:, : ncw * D], in0=acc[:, : ncw * D],
                    in1=tmp[:, : ncw * D], op=AO.bitwise_or)
                gbf_sb = gpool.tile([P, cpw * D], mybir.dt.bfloat16, tag="gbf")
                nc.scalar.activation(
                    gbf_sb[:, : ncw * D], acc[:, : ncw * D],
                    mybir.ActivationFunctionType.Copy,
                    bias=-16.0 * delta, scale=delta)
                mask_sb = mpool.tile([P, cpw * P], mybir.dt.bfloat16, tag="m")
                for j in range(ncw):
                    nc.vector.tensor_tensor(
                        out=mask_sb[:, j * P : (j + 1) * P],
                        in0=seg_sb[:, lo + j : lo + j + 1].to_broadcast([P, P]),
                        in1=iota_sb[:],
                        op=mybir.AluOpType.is_equal,
                    )
                psum = ppool.tile([P, D], mybir.dt.float32)
                for j in range(ncw):
                    nc.tensor.matmul(
                        out=psum[:],
                        lhsT=mask_sb[:, j * P : (j + 1) * P],
                        rhs=gbf_sb[:, j * D : (j + 1) * D],
                        start=(j == 0),
                        stop=(j == ncw - 1),
                    )
                stage = spool.tile([P, D], mybir.dt.bfloat16, tag="st")
                nc.scalar.copy(out=stage[:], in_=psum[:])
                nc.gpsimd.indirect_dma_start(
                    out=out_d[:],
                    out_offset=bass.IndirectOffsetOnAxis(
                        ap=obag_sb[:, w : w + 1], axis=0
                    ),
                    in_=stage[:],
                    in_offset=None,
                )

            # Consume the out-store DMAs so the tail drain stays under the
            # TPB_CTRL sync-wait limit: one readback touching the tensor.
            scrap = cpool.tile([P, 1], mybir.dt.bfloat16)
            nc.sync.dma_start(out=scrap[:, :], in_=out_d[0:P, 0:1])
    nc.finalize()
    return nc


def kernel(weights, values, offsets):
    global LAST_EXEC_NS, LAST_RESULTS
    weights = np.ascontiguousarray(np.asarray(weights), dtype=np.float32)
    values = np.asarray(values)
    offsets = np.asarray(offsets)
    vals32 = values.astype(np.int32)
    offs = offsets.astype(np.int64)

    # per-table bag id for every index position
    seg = np.empty((T_TABLES, L_IDX), np.int64)
    ar = np.arange(L_IDX)
    for t in range(T_TABLES):
        seg[t] = np.searchsorted(offs[t, 1:], ar, side="right")

    # greedy variable-length windows: extend while every table's bag span
    # stays <= 127 (so one 128-row psum block covers the window's bags)
    windows = []
    lo = 0
    while lo < NCHUNKS:
        hi = lo + 1
        while hi < NCHUNKS and hi - lo < MAX_CPW:
            if (seg[:, (hi + 1) * P - 1] - seg[:, lo * P]).max() > 127:
                break
            hi += 1
        windows.append((lo, hi))
        lo = hi
    for lo, hi in windows:  # safety: masks only cover local bags 0..127
        assert (seg[:, hi * P - 1] - seg[:, lo * P]).max() <= 127, \
            "pathological offsets: single window spans >128 bags"
    cpw = max(hi - lo for lo, hi in windows)
    W = len(windows)

    # dedup each table to its referenced rows; remap indices
    uniqs, invs = [], []
    for t in range(T_TABLES):
        u, inv = np.unique(vals32[t], return_inverse=True)
        uniqs.append(u)
        invs.append(inv.astype(np.int32))
    NU = max(u.size for u in uniqs)

    # one global symmetric 5-bit scale, baked into the program. Naive 5-bit
    # rounding is too lossy; instead each bag's exclusively-referenced rows
    # are rounded Floyd-Steinberg style, absorbing the bag's running
    # residual (including the plain-rounding errors of shared rows), so
    # per-bag quantization error telescopes to ~delta/2.
    delta = float(np.abs(weights).max()) / 15.0
    if delta == 0.0:
        delta = 1.0
    wp = np.zeros((T_TABLES, NU, PB), np.int8)
    for t in range(T_TABLES):
        nu = uniqs[t].size
        inv = invs[t]
        x = weights[t][uniqs[t]] * (1.0 / delta)          # [nu, D] ideal
        refcount = np.bincount(inv, minlength=nu)
        excl = refcount == 1
        w5 = np.clip(np.rint(x), -16, 15)                 # shared rows: naive
        e = x - w5
        carry = np.zeros((B_BAGS, D))
        sh_pos = np.nonzero(~excl[inv])[0]
        np.add.at(carry, seg[t][sh_pos], e[inv[sh_pos]])
        ex_pos = np.nonzero(excl[inv])[0]                 # bag-sorted order
        ex_rows = inv[ex_pos]
        ex_bags = seg[t][ex_pos]
        counts = np.bincount(ex_bags, minlength=B_BAGS)
        starts = np.concatenate([[0], np.cumsum(counts)[:-1]])
        resid = carry
        for j in range(int(counts.max())):                # step j of every bag
            act = np.nonzero(counts > j)[0]
            rows_j = ex_rows[starts[act] + j]
            tv = x[rows_j] + resid[act]
            q = np.clip(np.rint(tv), -16, 15)
            w5[rows_j] = q
            resid[act] = tv - q
        v = (w5 + 16.0).astype(np.uint8)                  # biased 0..31
        p4 = (v >> 1).astype(np.uint8)
        p1 = (v & 1).astype(np.uint8)
        b4 = (p4[:, 0::2] << 4) | p4[:, 1::2]
        b1 = np.zeros((nu, 16), np.uint8)
        for s in range(8):
            b1 |= p1[:, s::8] << (7 - s)
        wp[t, :nu] = np.concatenate([b4, b1], axis=1).astype(np.int8)

    assert NU <= (1 << 17), "row index does not fit 17 bits"
    first_bag = np.empty((T_TABLES, W), np.int64)
    gidx8 = np.empty((T_TABLES, P, 3 * NCHUNKS), np.uint8)
    obag = np.empty((T_TABLES, P, W), np.int32)
    rr = np.arange(P)
    for t in range(T_TABLES):
        fb = seg[t, [lo * P for lo, _ in windows]]
        first_bag[t] = fb
        fb_per_idx = np.repeat(fb, [(hi - lo) * P for lo, hi in windows])
        sl = seg[t] - fb_per_idx  # local bag id, 0..127
        assert sl.min() >= 0 and sl.max() <= 127
        packed = invs[t].astype(np.int64) | (sl.astype(np.int64) << 17)
        g32 = packed.astype(np.int32).reshape(NCHUNKS, P).T
        gidx8[t, :, 0:NCHUNKS] = g32 & 0xFF
        gidx8[t, :, NCHUNKS : 2 * NCHUNKS] = (g32 >> 8) & 0xFF
        gidx8[t, :, 2 * NCHUNKS : 3 * NCHUNKS] = (g32 >> 16) & 0xFF
        # scatter targets: final bag rows, boundary partial slot, dump row
        for w in range(W):
            bl = int(fb[w + 1] - fb[w]) if w + 1 < W else P
            col = np.where(
                (rr < bl) & (fb[w] + rr < B_BAGS), fb[w] + rr,
                np.where(rr == bl, B_BAGS + w, B_BAGS + W),
            )
            obag[t, :, w] = col

    global LAST_NC, LAST_INMAPS
    nc = _build_program(cpw, windows, NU, delta)
    in_maps = [
        {
            "wp": wp[t],
            "gidx": np.ascontiguousarray(gidx8[t]),
            "obag": np.ascontiguousarray(obag[t]),
        }
        for t in range(T_TABLES)
    ]
    LAST_NC, LAST_INMAPS = nc, in_maps
    import time as _time

    t0 = _time.time()
    res = run_bass_kernel_spmd(
        nc, in_maps, core_ids=list(range(T_TABLES)), trace=TRACE
    )
    first_s = _time.time() - t0
    LAST_EXEC_NS = res.exec_time_ns
    LAST_RESULTS = res
    if LAST_EXEC_NS is None and os.environ.get("EMB_TIME_RERUN", "1") == "1":
        # no NTFF hook in this container: re-execute the cached executable;
        # wall time upper-bounds kernel time (still includes input transfer).
        t0 = _time.time()
        res = run_bass_kernel_spmd(nc, in_maps, core_ids=list(range(T_TABLES)))
        LAST_EXEC_NS = int((_time.time() - t0) * 1e9)
        print(f"[kernel] first call {first_s:.1f}s, cached re-exec "
              f"{LAST_EXEC_NS/1e6:.1f}ms (incl. host<->device transfer)")

    big = np.empty((T_TABLES, B_BAGS, D), np.float32)
    for t in range(T_TABLES):
        out_t = res.results[t]["out"].astype(np.float32)
        big[t] = out_t[:B_BAGS]
        for w in range(W - 1):  # fold boundary partials into their bags
            big[t, int(first_bag[t, w + 1])] += out_t[B_BAGS + w]
    return big.transpose(1, 0, 2).reshape(B_BAGS, T_TABLES * D)
